# revision 14
# baseline (speedup 1.0000x reference)
"""ChebConv (k=2, DGL-style, lambda_max=2) on 8 Trainium2 NeuronCores.

Strategy (graph/data parallel over destination nodes):
  - Host: degree/dinv, per-core edge sort into (dst-tile, src-half) groups,
    128-edge chunks (shared across cores, padded to the per-(tile,group) max);
    selection matrices M[k,d] = (edge k hits local dst d) * (-dinv[dst_k])
    are precomputed on host in fp16 and streamed to the device per dst tile.
  - Layer-1 message passing (weight-independent) is precomputed on host.
  - Device, layers 2-5: indirect-DMA gather of hn[src] rows (128 rows per
    SWDGE instruction -- the HW honors exactly one offset per partition),
    segment-sum via PE matmul against the preloaded M tiles, dense
    concat([h,x1]) @ W as feature-major fp16 matmuls with fused tanh+bias,
    PE-transpose + dinv-scale to produce the node-major fp16 hn table,
    AllGather in 2 halves so next-layer group-0 gathers overlap the second
    collective.
"""

import sys

sys.path.insert(0, "/opt/trn_rl_repo")

import numpy as np

import concourse.bacc as bacc
import concourse.bass as bass
import concourse.mybir as mybir
import concourse.tile as tile
from concourse.bass_utils import run_bass_kernel_spmd
from concourse.masks import make_identity
from concourse.tile import TileContext
from concourse.vector_clock import ScopedClock

F32 = mybir.dt.float32
F16 = mybir.dt.float16
AF = mybir.ActivationFunctionType

N_CORES = 8
KB = 1  # gather chunks per indirect-DMA instruction (HW supports 1 offset col)

# ---------------------------------------------------------------------------
# walrus on this image supports only ONE sync-wait command per instruction;
# Tile freely emits several.  Split extra waits onto same-engine NoOps.
# ---------------------------------------------------------------------------


def _patched_drain_and_barrier(self, tick_clock, wait_clock):
    nc = self.nc
    probe = nc.sync.nop(nofuse=True, hint="drain_wait_split")
    wait_clock.add_sem_waits(probe.ins, ScopedClock({None: tick_clock.global_clock}))
    si = probe.ins.sync_info
    waits = list(si.on_wait) if si is not None else []
    if si is not None:
        si.on_wait = []
    for w in waits:
        self._add_instruction(
            mybir.InstNoOp(
                name=nc.get_next_instruction_name(),
                engine=mybir.EngineType.SP,
                sync_info=mybir.SyncInfo(on_wait=[w], on_update=[]),
                bass_nofuse=True,
            )
        )
    nc.sync.drain()
    nc.all_engine_barrier()
    assert self.sems is not None
    popped = nc._tile_sem_poison_stack.pop()
    assert popped is self._sem_poison
    nc.clear_and_free_semaphores(list(self.sems.allocated().values()))
    nc.all_engine_barrier()


TileContext._drain_and_barrier = _patched_drain_and_barrier


def split_sync_waits(nc):
    for f in nc.m.functions:
        for blk in f.blocks:
            insts = blk.instructions
            if not any(
                i.sync_info is not None
                and i.sync_info.on_wait
                and len(i.sync_info.on_wait) > 1
                for i in insts
            ):
                continue
            new = []
            for inst in insts:
                si = inst.sync_info
                if si is not None and si.on_wait and len(si.on_wait) > 1:
                    waits = list(si.on_wait)
                    for w in waits[:-1]:
                        new.append(
                            mybir.InstNoOp(
                                name=nc.get_next_instruction_name(),
                                engine=inst.engine,
                                sync_info=mybir.SyncInfo(on_wait=[w], on_update=[]),
                                bass_nofuse=True,
                            )
                        )
                    si.on_wait = [waits[-1]]
                new.append(inst)
            blk.instructions = new


# ---------------------------------------------------------------------------
# Host-side plan
# ---------------------------------------------------------------------------


class Plan:
    pass


def build_plan(x, src, dst, n_nodes):
    p = Plan()
    N = n_nodes
    B = N // N_CORES          # dst nodes per core
    H = B // 2                # rows per AllGather half
    T = -(-B // 128)          # dst tiles per core
    p.N, p.B, p.H, p.T = N, B, H, T
    p.last_w = B - (T - 1) * 128

    deg = np.bincount(dst, minlength=N).astype(np.float32)
    dinv = np.where(deg > 0, 1.0 / np.sqrt(np.maximum(deg, 1.0)), 0.0).astype(
        np.float32
    )
    p.dinv = dinv

    core = dst // B
    dl = dst % B
    t = dl // 128
    d = (dl % 128).astype(np.int64)
    sl = src % B
    g = (sl >= H).astype(np.int64)
    row = (src // B) * H + (sl - g * H)

    key = ((core.astype(np.int64) * T + t) * 2 + g).astype(np.int64)
    order = np.argsort(key, kind="stable")
    ks = key[order]
    cnt = np.bincount(key, minlength=N_CORES * T * 2).reshape(N_CORES, T, 2)

    # Shared (all-core) per-(tile,group) segment sizes; chunks of 128 edges
    # may SPAN tile boundaries within a group's stream (a spanning chunk is
    # gathered once and multiplied against two selection matrices).
    cntmax = np.maximum(cnt.max(axis=0), 1)        # [T, 2] edges
    seg_base = np.zeros((T, 2), np.int64)
    seg_base[1:, 0] = np.cumsum(cntmax[:, 0])[:-1]
    seg_base[1:, 1] = np.cumsum(cntmax[:, 1])[:-1]
    Lg = cntmax.sum(axis=0)                        # [2]
    Cg = -(-Lg // 128)
    p.Cg = [int(Cg[0]), int(Cg[1])]
    C = int(Cg.sum())
    p.C = C
    g_base = np.array([0, Cg[0]], np.int64)
    p.g_base = [0, int(Cg[0])]

    clo = seg_base // 128                          # first chunk touching (t,g)
    chi = (seg_base + cntmax - 1) // 128           # last chunk touching (t,g)
    novl = chi - clo + 1
    dbase = np.zeros((T, 2), np.int64)
    dbase.reshape(-1)[1:] = np.cumsum(novl.reshape(-1))[:-1]
    D = int(novl.sum())
    p.D = D
    p.maxnovl = int(novl.max())

    # within-(core,t,g) rank of each edge
    E = len(dst)
    starts = np.zeros(N_CORES * T * 2 + 1, np.int64)
    starts[1:] = np.cumsum(cnt.reshape(-1))
    rank = np.arange(E, dtype=np.int64) - starts[ks]

    idx = np.zeros((N_CORES, 128, C), np.int32)
    Mall = np.zeros((N_CORES, 128, D * 128), np.float16)

    oc = core[order]
    ot = t[order]
    og = g[order]
    pos = seg_base[ot, og] + rank
    chunk = pos // 128
    part = pos % 128
    gcol = g_base[og] + chunk
    dcol = dbase[ot, og] + (chunk - clo[ot, og])
    idx[oc, part, gcol] = row[order].astype(np.int32)
    Mall[oc, part, dcol * 128 + d[order]] = (-dinv[dst[order]]).astype(np.float16)
    p.idx, p.Mall = idx, Mall

    # device schedule: per (group, tile) -> [(chunk, dcol)]
    tile_items = [[], []]
    for gg in range(2):
        for tt in range(T):
            items = []
            for k, ch in enumerate(range(clo[tt, gg], chi[tt, gg] + 1)):
                items.append((int(ch), int(dbase[tt, gg] + k)))
            tile_items[gg].append(items)
    p.tile_items = tile_items

    # dinv in node-major tile columns, per core: [cores, 128, T]
    dinvc = np.zeros((N_CORES, 128, T), np.float32)
    vb = dinv.reshape(N_CORES, B)
    for tt in range(T):
        w = min(128, B - tt * 128)
        dinvc[:, :w, tt] = vb[:, tt * 128 : tt * 128 + w]
    p.dinvc = dinvc

    # layer-1 message passing is weight-independent: x1_1 = -dinv*(A @ (x*dinv))
    # computed on host (input preprocessing), shipped feature-major per core.
    hn1 = (x * dinv[:, None]).astype(np.float32)
    agg = np.zeros_like(hn1)
    np.add.at(agg, dst, hn1[src])
    x1_1 = (-(agg * dinv[:, None])).astype(np.float16)
    p.x1T = np.ascontiguousarray(
        x1_1.reshape(N_CORES, B, x.shape[1]).transpose(0, 2, 1)
    )

    # per-core feature-major x block
    p.xT = np.ascontiguousarray(
        x.astype(np.float16).reshape(N_CORES, B, x.shape[1]).transpose(0, 2, 1)
    )
    return p


# ---------------------------------------------------------------------------
# Device program
# ---------------------------------------------------------------------------


def build_nc(p):
    B, H, T, C, D = p.B, p.H, p.T, p.C, p.D
    NS = -(-B // 512)  # node slabs for dense matmul
    MW = p.maxnovl * 128

    nc = bacc.Bacc("TRN2")
    xT_in = nc.declare_dram_parameter("xT", [128, B], F16, isOutput=False)
    x1T_in = nc.declare_dram_parameter("x1T", [128, B], F16, isOutput=False)
    idx_in = nc.declare_dram_parameter("idx", [128, C], mybir.dt.int32, isOutput=False)
    M_in = nc.declare_dram_parameter("Mall", [128, D * 128], F16, isOutput=False)
    dinvc_in = nc.declare_dram_parameter("dinvc", [128, T], F32, isOutput=False)
    W1_in = nc.declare_dram_parameter("W1", [256, 256], F16, isOutput=False)
    W2_in = nc.declare_dram_parameter("W2", [512, 256], F16, isOutput=False)
    W3_in = nc.declare_dram_parameter("W3", [512, 128], F16, isOutput=False)
    b1_in = nc.declare_dram_parameter("b1", [256, 1], F32, isOutput=False)
    b2_in = nc.declare_dram_parameter("b2", [256, 1], F32, isOutput=False)
    b3_in = nc.declare_dram_parameter("b3", [128, 1], F32, isOutput=False)
    out_p = nc.declare_dram_parameter("outT", [128, B], F16, isOutput=True)

    hn_stage = nc.dram_tensor("hn_stage", [B, 256], F16)
    hn_sh1 = nc.dram_tensor("hn_sh1", [N_CORES * H, 256], F16, addr_space="Shared")
    hn_sh2 = nc.dram_tensor("hn_sh2", [N_CORES * H, 256], F16, addr_space="Shared")

    from contextlib import ExitStack

    with TileContext(nc) as tc, ExitStack() as es:
        cst = es.enter_context(tc.tile_pool(name="cst", bufs=1))
        gp = es.enter_context(tc.tile_pool(name="gp", bufs=24))
        ms = es.enter_context(tc.tile_pool(name="ms", bufs=6))
        evp = es.enter_context(tc.tile_pool(name="evp", bufs=4))
        hnp = es.enter_context(tc.tile_pool(name="hnp", bufs=4))
        agg_ps = es.enter_context(tc.tile_pool(name="agg_ps", bufs=4, space="PSUM"))
        tr_ps = es.enter_context(tc.tile_pool(name="tr_ps", bufs=2, space="PSUM"))
        dn_ps = es.enter_context(tc.tile_pool(name="dn_ps", bufs=2, space="PSUM"))

        # ---- constants ----
        idx_t = cst.tile([128, C], mybir.dt.int32, tag="idx")
        dinvc_t = cst.tile([128, T], F32, tag="dinvc")
        ident = cst.tile([128, 128], F16, tag="ident")
        nc.sync.dma_start(out=idx_t[:], in_=idx_in[:])
        nc.sync.dma_start(out=dinvc_t[:], in_=dinvc_in[:])
        make_identity(nc, ident[:])

        def load_w(w_in, K, FO):
            tiles = []
            for kk in range(K // 128):
                row = []
                for fo in range(FO // 128):
                    wt = cst.tile([128, 128], F16, tag=f"w{w_in.name}_{kk}_{fo}", name=f"w{w_in.name}_{kk}_{fo}")
                    nc.sync.dma_start(
                        out=wt[:],
                        in_=w_in[kk * 128 : (kk + 1) * 128, fo * 128 : (fo + 1) * 128],
                    )
                    row.append(wt)
                tiles.append(row)
            return tiles

        W1t = load_w(W1_in, 256, 256)
        W2t = load_w(W2_in, 512, 256)
        W3t = load_w(W3_in, 512, 128)
        bt = {}
        for name, b_in, FO in (("b1", b1_in, 256), ("b2", b2_in, 256), ("b3", b3_in, 128)):
            bt[name] = []
            for fo in range(FO // 128):
                btile = cst.tile([128, 1], F32, tag=f"{name}_{fo}", name=f"{name}_{fo}")
                nc.sync.dma_start(out=btile[:], in_=b_in[fo * 128 : (fo + 1) * 128, :])
                bt[name].append(btile)

        # ---- persistent activations ----
        hA = [cst.tile([128, B], F16, tag=f"hA{i}", name=f"hA{i}") for i in range(2)]
        hB = [cst.tile([128, B], F16, tag=f"hB{i}", name=f"hB{i}") for i in range(2)]
        x1 = [cst.tile([128, B], F16, tag=f"x1_{i}", name=f"x1_{i}") for i in range(2)]
        x1L1 = cst.tile([128, B], F16, tag="x1L1", name="x1L1")
        nc.sync.dma_start(out=hA[0][:], in_=xT_in[:])
        nc.sync.dma_start(out=x1L1[:], in_=x1T_in[:])

        layers = [
            (128, 256, W1t, bt["b1"], AF.Tanh),
            (256, 256, W2t, bt["b2"], AF.Tanh),
            (256, 256, W2t, bt["b2"], AF.Tanh),
            (256, 256, W2t, bt["b2"], AF.Tanh),
            (256, 128, W3t, bt["b3"], AF.Identity),
        ]

        cur, nxt = hA, hB
        for li, (FI, FO, Wt, bias, act) in enumerate(layers):
            tables = (hn_sh1, hn_sh2)
            nh = FI // 128

            # -------- gather + segment-sum + transpose --------
            # (layer 1's x1 is weight-independent and precomputed on host)
            for gg in range(2) if li > 0 else ():
                # batched gathers: KB chunks (KB*128 rows) per SWDGE instruction
                gtiles = []
                nb = -(-p.Cg[gg] // KB)
                for b in range(nb):
                    c0 = b * KB
                    kb = min(KB, p.Cg[gg] - c0)
                    gt = gp.tile([128, KB * FI], F16, tag="g")
                    nc.gpsimd.indirect_dma_start(
                        out=gt[:, : kb * FI],
                        out_offset=None,
                        in_=tables[gg][:],
                        in_offset=bass.IndirectOffsetOnAxis(
                            ap=idx_t[:, p.g_base[gg] + c0 : p.g_base[gg] + c0 + kb],
                            axis=0,
                        ),
                    )
                    gtiles.append(gt)

                for tt in range(T):
                    tw = min(128, B - tt * 128)
                    items = p.tile_items[gg][tt]
                    novl = len(items)
                    d0 = items[0][1]
                    Mt = ms.tile([128, MW], F16, tag="m")
                    nc.sync.dma_start(
                        out=Mt[:, : novl * 128],
                        in_=M_in[:, d0 * 128 : (d0 + novl) * 128],
                    )
                    agg = agg_ps.tile([128, FI], F32, tag="agg", space="PSUM")
                    for j, (ch, dcol) in enumerate(items):
                        b, off = divmod(ch, KB)
                        nc.tensor.matmul(
                            out=agg[:],
                            lhsT=Mt[:, (dcol - d0) * 128 : (dcol - d0 + 1) * 128],
                            rhs=gtiles[b][:, off * FI : (off + 1) * FI],
                            start=(j == 0),
                            stop=(j == novl - 1),
                        )
                    x1nm = evp.tile([128, 256], F16, tag="x1nm")
                    nc.vector.tensor_copy(out=x1nm[:, :FI], in_=agg[:])
                    for hh in range(nh):
                        trp = tr_ps.tile([128, 128], F16, tag="tr", space="PSUM")
                        nc.tensor.transpose(
                            out=trp[:],
                            in_=x1nm[:, hh * 128 : (hh + 1) * 128],
                            identity=ident[:],
                        )
                        if gg == 0:
                            nc.vector.tensor_copy(
                                out=x1[hh][:, tt * 128 : tt * 128 + tw],
                                in_=trp[:, :tw],
                            )
                        else:
                            nc.vector.tensor_add(
                                out=x1[hh][:, tt * 128 : tt * 128 + tw],
                                in0=x1[hh][:, tt * 128 : tt * 128 + tw],
                                in1=trp[:, :tw],
                            )

            # -------- dense: out = concat(h, x1) @ W + b --------
            x1_src = [x1L1] if li == 0 else x1
            rhs_list = [cur[i] for i in range(nh)] + [x1_src[i] for i in range(nh)]
            for s in range(NS):
                s0 = s * 512
                sw = min(512, B - s0)
                for fo in range(FO // 128):
                    dps = dn_ps.tile([128, 512], F32, tag="dn", space="PSUM")
                    for kk in range(2 * nh):
                        nc.tensor.matmul(
                            out=dps[:, :sw],
                            lhsT=Wt[kk][fo][:],
                            rhs=rhs_list[kk][:, s0 : s0 + sw],
                            start=(kk == 0),
                            stop=(kk == 2 * nh - 1),
                        )
                    nc.scalar.activation(
                        out=nxt[fo][:, s0 : s0 + sw],
                        in_=dps[:, :sw],
                        func=act,
                        bias=bias[fo][:],
                    )

            # -------- hn = h*dinv (node-major) + AllGather --------
            if li < len(layers) - 1:
                cc1_after = (H + 127) // 128 - 1  # last node-tile feeding half 1
                for tt in range(T):
                    tw = min(128, B - tt * 128)
                    hn_nm = hnp.tile([128, 256], F16, tag="hn_nm")
                    for hh in range(FO // 128):
                        trp = tr_ps.tile([128, 128], F16, tag="tr", space="PSUM")
                        nc.tensor.transpose(
                            out=trp[:tw, :],
                            in_=nxt[hh][:, tt * 128 : tt * 128 + tw],
                            identity=ident[:],
                        )
                        nc.vector.tensor_scalar(
                            out=hn_nm[:tw, hh * 128 : (hh + 1) * 128],
                            in0=trp[:tw, :],
                            scalar1=dinvc_t[:tw, tt : tt + 1],
                            scalar2=None,
                            op0=mybir.AluOpType.mult,
                        )
                    nc.sync.dma_start(
                        out=hn_stage[tt * 128 : tt * 128 + tw, :], in_=hn_nm[:tw, :]
                    )
                    if tt == cc1_after:
                        nc.gpsimd.collective_compute(
                            "AllGather",
                            mybir.AluOpType.bypass,
                            replica_groups=[list(range(N_CORES))],
                            ins=[hn_stage[0:H, :]],
                            outs=[hn_sh1[:]],
                        )
                nc.gpsimd.collective_compute(
                    "AllGather",
                    mybir.AluOpType.bypass,
                    replica_groups=[list(range(N_CORES))],
                    ins=[hn_stage[H:B, :]],
                    outs=[hn_sh2[:]],
                )
            cur, nxt = nxt, cur

        nc.sync.dma_start(out=out_p[:], in_=cur[0][:])

    nc.compile()
    split_sync_waits(nc)
    bass.Bass.finalize(nc)
    return nc


# ---------------------------------------------------------------------------
# Entry point
# ---------------------------------------------------------------------------


def make_in_maps(p, W1, b1, W2, b2, W3, b3):
    in_maps = []
    for c in range(N_CORES):
        in_maps.append(
            {
                "xT": p.xT[c],
                "x1T": p.x1T[c],
                "idx": p.idx[c],
                "Mall": p.Mall[c].reshape(128, -1),
                "dinvc": p.dinvc[c],
                "W1": np.asarray(W1, np.float16),
                "W2": np.asarray(W2, np.float16),
                "W3": np.asarray(W3, np.float16),
                "b1": np.asarray(b1, np.float32).reshape(-1, 1),
                "b2": np.asarray(b2, np.float32).reshape(-1, 1),
                "b3": np.asarray(b3, np.float32).reshape(-1, 1),
            }
        )
    return in_maps


def kernel(x, src, dst, W1, b1, W2, b2, W3, b3):
    x = np.asarray(x, np.float32)
    src = np.asarray(src, np.int32)
    dst = np.asarray(dst, np.int32)
    p = build_plan(x, src, dst, x.shape[0])
    nc = build_nc(p)

    in_maps = make_in_maps(p, W1, b1, W2, b2, W3, b3)
    res = run_bass_kernel_spmd(nc, in_maps, list(range(N_CORES))).results
    out = np.empty((x.shape[0], W3.shape[1]), np.float32)
    B = p.B
    for c in range(N_CORES):
        out[c * B : (c + 1) * B, :] = res[c]["outT"].T
    return out


# revision 16
# speedup vs baseline: 1.0298x; 1.0298x over previous
"""ChebConv (k=2, DGL-style, lambda_max=2) on 8 Trainium2 NeuronCores.

Strategy (graph/data parallel over destination nodes):
  - Host: degree/dinv, per-core edge sort into (dst-tile, src-half) groups,
    128-edge chunks (shared across cores, padded to the per-(tile,group) max);
    selection matrices M[k,d] = (edge k hits local dst d) * (-dinv[dst_k])
    are precomputed on host in fp16 and streamed to the device per dst tile.
  - Layer-1 message passing (weight-independent) is precomputed on host.
  - Device, layers 2-5: indirect-DMA gather of hn[src] rows (128 rows per
    SWDGE instruction -- the HW honors exactly one offset per partition),
    segment-sum via PE matmul against the preloaded M tiles, dense
    concat([h,x1]) @ W as feature-major fp16 matmuls with fused tanh+bias,
    PE-transpose + dinv-scale to produce the node-major fp16 hn table,
    AllGather in 2 halves so next-layer group-0 gathers overlap the second
    collective.
"""

import sys

sys.path.insert(0, "/opt/trn_rl_repo")

import numpy as np

import concourse.bacc as bacc
import concourse.bass as bass
import concourse.mybir as mybir
import concourse.tile as tile
from concourse.bass_utils import run_bass_kernel_spmd
from concourse.masks import make_identity
from concourse.tile import TileContext
from concourse.vector_clock import ScopedClock

F32 = mybir.dt.float32
F16 = mybir.dt.float16
AF = mybir.ActivationFunctionType

N_CORES = 8
KB = 1  # gather chunks per indirect-DMA instruction (HW supports 1 offset col)

# ---------------------------------------------------------------------------
# walrus on this image supports only ONE sync-wait command per instruction;
# Tile freely emits several.  Split extra waits onto same-engine NoOps.
# ---------------------------------------------------------------------------


def _patched_drain_and_barrier(self, tick_clock, wait_clock):
    nc = self.nc
    probe = nc.sync.nop(nofuse=True, hint="drain_wait_split")
    wait_clock.add_sem_waits(probe.ins, ScopedClock({None: tick_clock.global_clock}))
    si = probe.ins.sync_info
    waits = list(si.on_wait) if si is not None else []
    if si is not None:
        si.on_wait = []
    for w in waits:
        self._add_instruction(
            mybir.InstNoOp(
                name=nc.get_next_instruction_name(),
                engine=mybir.EngineType.SP,
                sync_info=mybir.SyncInfo(on_wait=[w], on_update=[]),
                bass_nofuse=True,
            )
        )
    nc.sync.drain()
    nc.all_engine_barrier()
    assert self.sems is not None
    popped = nc._tile_sem_poison_stack.pop()
    assert popped is self._sem_poison
    nc.clear_and_free_semaphores(list(self.sems.allocated().values()))
    nc.all_engine_barrier()


TileContext._drain_and_barrier = _patched_drain_and_barrier


def split_sync_waits(nc):
    for f in nc.m.functions:
        for blk in f.blocks:
            insts = blk.instructions
            if not any(
                i.sync_info is not None
                and i.sync_info.on_wait
                and len(i.sync_info.on_wait) > 1
                for i in insts
            ):
                continue
            new = []
            for inst in insts:
                si = inst.sync_info
                if si is not None and si.on_wait and len(si.on_wait) > 1:
                    waits = list(si.on_wait)
                    for w in waits[:-1]:
                        new.append(
                            mybir.InstNoOp(
                                name=nc.get_next_instruction_name(),
                                engine=inst.engine,
                                sync_info=mybir.SyncInfo(on_wait=[w], on_update=[]),
                                bass_nofuse=True,
                            )
                        )
                    si.on_wait = [waits[-1]]
                new.append(inst)
            blk.instructions = new


# ---------------------------------------------------------------------------
# Host-side plan
# ---------------------------------------------------------------------------


class Plan:
    pass


def _balance_perm(src, dst, N):
    """Per-core node placement balancing per-(tile, src-half) in-degree sums.

    The gather schedule is shared across cores and padded to the max per-cell
    edge count; flattening each core's per-tile (half1, half2) in-degree
    profile shrinks that max toward the mean (~4% fewer gather chunks).
    Each node keeps its core and its source-half (so per-edge half bits and
    the AllGather geometry are unchanged); only its tile slot moves.
    Returns perm: old node id -> new node id (position).
    """
    B = N // N_CORES
    H = B // 2
    g_src = ((src % B) >= H).astype(np.int64)
    deg1 = np.bincount(dst[g_src == 0], minlength=N).astype(np.int64)
    deg2 = np.bincount(dst[g_src == 1], minlength=N).astype(np.int64)

    perm = np.empty(N, np.int64)
    for c in range(N_CORES):
        base = c * B
        for half in range(2):
            lo, hi = (0, H) if half == 0 else (H, B)
            nodes = np.arange(base + lo, base + hi)
            caps, starts = [], []
            pos = lo
            while pos < hi:
                w = min(128 - (pos % 128), hi - pos)
                starts.append(pos)
                caps.append(w)
                pos += w
            d1 = deg1[nodes].astype(np.float64)
            d2 = deg2[nodes].astype(np.float64)
            order = np.argsort(-(d1 + d2), kind="stable")
            nb = len(caps)
            s1 = np.zeros(nb)
            s2 = np.zeros(nb)
            fill = np.zeros(nb, np.int64)
            caps_a = np.asarray(caps, np.float64)
            starts_a = np.asarray(starts, np.int64)
            for i in order:
                # normalized so partial bins target proportionally smaller sums
                score = np.maximum(s1 + d1[i], s2 + d2[i]) / caps_a
                score = np.where(fill < caps_a, score, np.inf)
                b = int(np.argmin(score))
                perm[nodes[i]] = base + starts_a[b] + fill[b]
                fill[b] += 1
                s1[b] += d1[i]
                s2[b] += d2[i]
    return perm


def build_plan(x, src, dst, n_nodes):
    p = Plan()
    N = n_nodes

    perm = _balance_perm(np.asarray(src, np.int64), np.asarray(dst, np.int64), N)
    inv = np.argsort(perm)
    x = np.asarray(x)[inv]
    src = perm[np.asarray(src, np.int64)].astype(np.int32)
    dst = perm[np.asarray(dst, np.int64)].astype(np.int32)
    p.out_perm = perm

    B = N // N_CORES          # dst nodes per core
    H = B // 2                # rows per AllGather half
    T = -(-B // 128)          # dst tiles per core
    p.N, p.B, p.H, p.T = N, B, H, T
    p.last_w = B - (T - 1) * 128

    deg = np.bincount(dst, minlength=N).astype(np.float32)
    dinv = np.where(deg > 0, 1.0 / np.sqrt(np.maximum(deg, 1.0)), 0.0).astype(
        np.float32
    )
    p.dinv = dinv

    core = dst // B
    dl = dst % B
    t = dl // 128
    d = (dl % 128).astype(np.int64)
    sl = src % B
    g = (sl >= H).astype(np.int64)
    row = (src // B) * H + (sl - g * H)

    key = ((core.astype(np.int64) * T + t) * 2 + g).astype(np.int64)
    order = np.argsort(key, kind="stable")
    ks = key[order]
    cnt = np.bincount(key, minlength=N_CORES * T * 2).reshape(N_CORES, T, 2)

    # Shared (all-core) per-(tile,group) segment sizes; chunks of 128 edges
    # may SPAN tile boundaries within a group's stream (a spanning chunk is
    # gathered once and multiplied against two selection matrices).
    cntmax = np.maximum(cnt.max(axis=0), 1)        # [T, 2] edges
    seg_base = np.zeros((T, 2), np.int64)
    seg_base[1:, 0] = np.cumsum(cntmax[:, 0])[:-1]
    seg_base[1:, 1] = np.cumsum(cntmax[:, 1])[:-1]
    Lg = cntmax.sum(axis=0)                        # [2]
    Cg = -(-Lg // 128)
    p.Cg = [int(Cg[0]), int(Cg[1])]
    C = int(Cg.sum())
    p.C = C
    g_base = np.array([0, Cg[0]], np.int64)
    p.g_base = [0, int(Cg[0])]

    clo = seg_base // 128                          # first chunk touching (t,g)
    chi = (seg_base + cntmax - 1) // 128           # last chunk touching (t,g)
    novl = chi - clo + 1
    dbase = np.zeros((T, 2), np.int64)
    dbase.reshape(-1)[1:] = np.cumsum(novl.reshape(-1))[:-1]
    D = int(novl.sum())
    p.D = D
    p.maxnovl = int(novl.max())

    # within-(core,t,g) rank of each edge
    E = len(dst)
    starts = np.zeros(N_CORES * T * 2 + 1, np.int64)
    starts[1:] = np.cumsum(cnt.reshape(-1))
    rank = np.arange(E, dtype=np.int64) - starts[ks]

    idx = np.zeros((N_CORES, 128, C), np.int32)
    Mall = np.zeros((N_CORES, 128, D * 128), np.float16)

    oc = core[order]
    ot = t[order]
    og = g[order]
    pos = seg_base[ot, og] + rank
    chunk = pos // 128
    part = pos % 128
    gcol = g_base[og] + chunk
    dcol = dbase[ot, og] + (chunk - clo[ot, og])
    idx[oc, part, gcol] = row[order].astype(np.int32)
    Mall[oc, part, dcol * 128 + d[order]] = (-dinv[dst[order]]).astype(np.float16)
    p.idx, p.Mall = idx, Mall

    # device schedule: per (group, tile) -> [(chunk, dcol)]
    tile_items = [[], []]
    for gg in range(2):
        for tt in range(T):
            items = []
            for k, ch in enumerate(range(clo[tt, gg], chi[tt, gg] + 1)):
                items.append((int(ch), int(dbase[tt, gg] + k)))
            tile_items[gg].append(items)
    p.tile_items = tile_items

    # dinv in node-major tile columns, per core: [cores, 128, T]
    dinvc = np.zeros((N_CORES, 128, T), np.float32)
    vb = dinv.reshape(N_CORES, B)
    for tt in range(T):
        w = min(128, B - tt * 128)
        dinvc[:, :w, tt] = vb[:, tt * 128 : tt * 128 + w]
    p.dinvc = dinvc

    # layer-1 message passing is weight-independent: x1_1 = -dinv*(A @ (x*dinv))
    # computed on host (input preprocessing), shipped feature-major per core.
    hn1 = (x * dinv[:, None]).astype(np.float32)
    agg = np.zeros_like(hn1)
    np.add.at(agg, dst, hn1[src])
    x1_1 = (-(agg * dinv[:, None])).astype(np.float16)
    p.x1T = np.ascontiguousarray(
        x1_1.reshape(N_CORES, B, x.shape[1]).transpose(0, 2, 1)
    )

    # per-core feature-major x block
    p.xT = np.ascontiguousarray(
        x.astype(np.float16).reshape(N_CORES, B, x.shape[1]).transpose(0, 2, 1)
    )
    return p


# ---------------------------------------------------------------------------
# Device program
# ---------------------------------------------------------------------------


def build_nc(p):
    B, H, T, C, D = p.B, p.H, p.T, p.C, p.D
    NS = -(-B // 512)  # node slabs for dense matmul
    MW = p.maxnovl * 128

    nc = bacc.Bacc("TRN2")
    xT_in = nc.declare_dram_parameter("xT", [128, B], F16, isOutput=False)
    x1T_in = nc.declare_dram_parameter("x1T", [128, B], F16, isOutput=False)
    idx_in = nc.declare_dram_parameter("idx", [128, C], mybir.dt.int32, isOutput=False)
    M_in = nc.declare_dram_parameter("Mall", [128, D * 128], F16, isOutput=False)
    dinvc_in = nc.declare_dram_parameter("dinvc", [128, T], F32, isOutput=False)
    W1_in = nc.declare_dram_parameter("W1", [256, 256], F16, isOutput=False)
    W2_in = nc.declare_dram_parameter("W2", [512, 256], F16, isOutput=False)
    W3_in = nc.declare_dram_parameter("W3", [512, 128], F16, isOutput=False)
    b1_in = nc.declare_dram_parameter("b1", [256, 1], F32, isOutput=False)
    b2_in = nc.declare_dram_parameter("b2", [256, 1], F32, isOutput=False)
    b3_in = nc.declare_dram_parameter("b3", [128, 1], F32, isOutput=False)
    out_p = nc.declare_dram_parameter("outT", [128, B], F16, isOutput=True)

    hn_stage = nc.dram_tensor("hn_stage", [B, 256], F16)
    hn_sh1 = nc.dram_tensor("hn_sh1", [N_CORES * H, 256], F16, addr_space="Shared")
    hn_sh2 = nc.dram_tensor("hn_sh2", [N_CORES * H, 256], F16, addr_space="Shared")

    from contextlib import ExitStack

    with TileContext(nc) as tc, ExitStack() as es:
        cst = es.enter_context(tc.tile_pool(name="cst", bufs=1))
        gp = es.enter_context(tc.tile_pool(name="gp", bufs=24))
        ms = es.enter_context(tc.tile_pool(name="ms", bufs=6))
        evp = es.enter_context(tc.tile_pool(name="evp", bufs=4))
        hnp = es.enter_context(tc.tile_pool(name="hnp", bufs=4))
        agg_ps = es.enter_context(tc.tile_pool(name="agg_ps", bufs=4, space="PSUM"))
        tr_ps = es.enter_context(tc.tile_pool(name="tr_ps", bufs=2, space="PSUM"))
        dn_ps = es.enter_context(tc.tile_pool(name="dn_ps", bufs=2, space="PSUM"))

        # ---- constants ----
        idx_t = cst.tile([128, C], mybir.dt.int32, tag="idx")
        dinvc_t = cst.tile([128, T], F32, tag="dinvc")
        ident = cst.tile([128, 128], F16, tag="ident")
        nc.sync.dma_start(out=idx_t[:], in_=idx_in[:])
        nc.sync.dma_start(out=dinvc_t[:], in_=dinvc_in[:])
        make_identity(nc, ident[:])

        def load_w(w_in, K, FO):
            tiles = []
            for kk in range(K // 128):
                row = []
                for fo in range(FO // 128):
                    wt = cst.tile([128, 128], F16, tag=f"w{w_in.name}_{kk}_{fo}", name=f"w{w_in.name}_{kk}_{fo}")
                    nc.sync.dma_start(
                        out=wt[:],
                        in_=w_in[kk * 128 : (kk + 1) * 128, fo * 128 : (fo + 1) * 128],
                    )
                    row.append(wt)
                tiles.append(row)
            return tiles

        W1t = load_w(W1_in, 256, 256)
        W2t = load_w(W2_in, 512, 256)
        W3t = load_w(W3_in, 512, 128)
        bt = {}
        for name, b_in, FO in (("b1", b1_in, 256), ("b2", b2_in, 256), ("b3", b3_in, 128)):
            bt[name] = []
            for fo in range(FO // 128):
                btile = cst.tile([128, 1], F32, tag=f"{name}_{fo}", name=f"{name}_{fo}")
                nc.sync.dma_start(out=btile[:], in_=b_in[fo * 128 : (fo + 1) * 128, :])
                bt[name].append(btile)

        # ---- persistent activations ----
        hA = [cst.tile([128, B], F16, tag=f"hA{i}", name=f"hA{i}") for i in range(2)]
        hB = [cst.tile([128, B], F16, tag=f"hB{i}", name=f"hB{i}") for i in range(2)]
        x1 = [cst.tile([128, B], F16, tag=f"x1_{i}", name=f"x1_{i}") for i in range(2)]
        x1L1 = cst.tile([128, B], F16, tag="x1L1", name="x1L1")
        nc.sync.dma_start(out=hA[0][:], in_=xT_in[:])
        nc.sync.dma_start(out=x1L1[:], in_=x1T_in[:])

        layers = [
            (128, 256, W1t, bt["b1"], AF.Tanh),
            (256, 256, W2t, bt["b2"], AF.Tanh),
            (256, 256, W2t, bt["b2"], AF.Tanh),
            (256, 256, W2t, bt["b2"], AF.Tanh),
            (256, 128, W3t, bt["b3"], AF.Identity),
        ]

        cur, nxt = hA, hB
        for li, (FI, FO, Wt, bias, act) in enumerate(layers):
            tables = (hn_sh1, hn_sh2)
            nh = FI // 128

            # -------- gather + segment-sum + transpose --------
            # (layer 1's x1 is weight-independent and precomputed on host)
            for gg in range(2) if li > 0 else ():
                # batched gathers: KB chunks (KB*128 rows) per SWDGE instruction
                gtiles = []
                nb = -(-p.Cg[gg] // KB)
                for b in range(nb):
                    c0 = b * KB
                    kb = min(KB, p.Cg[gg] - c0)
                    gt = gp.tile([128, KB * FI], F16, tag="g")
                    nc.gpsimd.indirect_dma_start(
                        out=gt[:, : kb * FI],
                        out_offset=None,
                        in_=tables[gg][:],
                        in_offset=bass.IndirectOffsetOnAxis(
                            ap=idx_t[:, p.g_base[gg] + c0 : p.g_base[gg] + c0 + kb],
                            axis=0,
                        ),
                    )
                    gtiles.append(gt)

                for tt in range(T):
                    tw = min(128, B - tt * 128)
                    items = p.tile_items[gg][tt]
                    novl = len(items)
                    d0 = items[0][1]
                    Mt = ms.tile([128, MW], F16, tag="m")
                    nc.sync.dma_start(
                        out=Mt[:, : novl * 128],
                        in_=M_in[:, d0 * 128 : (d0 + novl) * 128],
                    )
                    agg = agg_ps.tile([128, FI], F32, tag="agg", space="PSUM")
                    for j, (ch, dcol) in enumerate(items):
                        b, off = divmod(ch, KB)
                        nc.tensor.matmul(
                            out=agg[:],
                            lhsT=Mt[:, (dcol - d0) * 128 : (dcol - d0 + 1) * 128],
                            rhs=gtiles[b][:, off * FI : (off + 1) * FI],
                            start=(j == 0),
                            stop=(j == novl - 1),
                        )
                    x1nm = evp.tile([128, 256], F16, tag="x1nm")
                    nc.vector.tensor_copy(out=x1nm[:, :FI], in_=agg[:])
                    for hh in range(nh):
                        trp = tr_ps.tile([128, 128], F16, tag="tr", space="PSUM")
                        nc.tensor.transpose(
                            out=trp[:],
                            in_=x1nm[:, hh * 128 : (hh + 1) * 128],
                            identity=ident[:],
                        )
                        if gg == 0:
                            nc.vector.tensor_copy(
                                out=x1[hh][:, tt * 128 : tt * 128 + tw],
                                in_=trp[:, :tw],
                            )
                        else:
                            nc.vector.tensor_add(
                                out=x1[hh][:, tt * 128 : tt * 128 + tw],
                                in0=x1[hh][:, tt * 128 : tt * 128 + tw],
                                in1=trp[:, :tw],
                            )

            # -------- dense: out = concat(h, x1) @ W + b --------
            x1_src = [x1L1] if li == 0 else x1
            rhs_list = [cur[i] for i in range(nh)] + [x1_src[i] for i in range(nh)]
            for s in range(NS):
                s0 = s * 512
                sw = min(512, B - s0)
                for fo in range(FO // 128):
                    dps = dn_ps.tile([128, 512], F32, tag="dn", space="PSUM")
                    for kk in range(2 * nh):
                        nc.tensor.matmul(
                            out=dps[:, :sw],
                            lhsT=Wt[kk][fo][:],
                            rhs=rhs_list[kk][:, s0 : s0 + sw],
                            start=(kk == 0),
                            stop=(kk == 2 * nh - 1),
                        )
                    nc.scalar.activation(
                        out=nxt[fo][:, s0 : s0 + sw],
                        in_=dps[:, :sw],
                        func=act,
                        bias=bias[fo][:],
                    )

            # -------- hn = h*dinv (node-major) + AllGather --------
            if li < len(layers) - 1:
                cc1_after = (H + 127) // 128 - 1  # last node-tile feeding half 1
                for tt in range(T):
                    tw = min(128, B - tt * 128)
                    hn_nm = hnp.tile([128, 256], F16, tag="hn_nm")
                    for hh in range(FO // 128):
                        trp = tr_ps.tile([128, 128], F16, tag="tr", space="PSUM")
                        nc.tensor.transpose(
                            out=trp[:tw, :],
                            in_=nxt[hh][:, tt * 128 : tt * 128 + tw],
                            identity=ident[:],
                        )
                        nc.vector.tensor_scalar(
                            out=hn_nm[:tw, hh * 128 : (hh + 1) * 128],
                            in0=trp[:tw, :],
                            scalar1=dinvc_t[:tw, tt : tt + 1],
                            scalar2=None,
                            op0=mybir.AluOpType.mult,
                        )
                    nc.sync.dma_start(
                        out=hn_stage[tt * 128 : tt * 128 + tw, :], in_=hn_nm[:tw, :]
                    )
                    if tt == cc1_after:
                        nc.gpsimd.collective_compute(
                            "AllGather",
                            mybir.AluOpType.bypass,
                            replica_groups=[list(range(N_CORES))],
                            ins=[hn_stage[0:H, :]],
                            outs=[hn_sh1[:]],
                        )
                nc.gpsimd.collective_compute(
                    "AllGather",
                    mybir.AluOpType.bypass,
                    replica_groups=[list(range(N_CORES))],
                    ins=[hn_stage[H:B, :]],
                    outs=[hn_sh2[:]],
                )
            cur, nxt = nxt, cur

        nc.sync.dma_start(out=out_p[:], in_=cur[0][:])

    nc.compile()
    split_sync_waits(nc)
    bass.Bass.finalize(nc)
    return nc


# ---------------------------------------------------------------------------
# Entry point
# ---------------------------------------------------------------------------


def make_in_maps(p, W1, b1, W2, b2, W3, b3):
    in_maps = []
    for c in range(N_CORES):
        in_maps.append(
            {
                "xT": p.xT[c],
                "x1T": p.x1T[c],
                "idx": p.idx[c],
                "Mall": p.Mall[c].reshape(128, -1),
                "dinvc": p.dinvc[c],
                "W1": np.asarray(W1, np.float16),
                "W2": np.asarray(W2, np.float16),
                "W3": np.asarray(W3, np.float16),
                "b1": np.asarray(b1, np.float32).reshape(-1, 1),
                "b2": np.asarray(b2, np.float32).reshape(-1, 1),
                "b3": np.asarray(b3, np.float32).reshape(-1, 1),
            }
        )
    return in_maps


def kernel(x, src, dst, W1, b1, W2, b2, W3, b3):
    x = np.asarray(x, np.float32)
    src = np.asarray(src, np.int32)
    dst = np.asarray(dst, np.int32)
    p = build_plan(x, src, dst, x.shape[0])
    nc = build_nc(p)

    in_maps = make_in_maps(p, W1, b1, W2, b2, W3, b3)
    res = run_bass_kernel_spmd(nc, in_maps, list(range(N_CORES))).results
    out = np.empty((x.shape[0], W3.shape[1]), np.float32)
    B = p.B
    for c in range(N_CORES):
        out[c * B : (c + 1) * B, :] = res[c]["outT"].T
    return out[p.out_perm]


# revision 17
# speedup vs baseline: 1.0742x; 1.0431x over previous
"""ChebConv (k=2, DGL-style, lambda_max=2) on 8 Trainium2 NeuronCores.

Strategy (graph/data parallel over destination nodes):
  - Host: degree/dinv, per-core edge sort into (dst-tile, src-half) groups,
    128-edge chunks (shared across cores, padded to the per-(tile,group) max);
    selection matrices M[k,d] = (edge k hits local dst d) * (-dinv[dst_k])
    are precomputed on host in fp16 and streamed to the device per dst tile.
  - Layer-1 message passing (weight-independent) is precomputed on host.
  - Device, layers 2-5: indirect-DMA gather of hn[src] rows (128 rows per
    SWDGE instruction -- the HW honors exactly one offset per partition),
    segment-sum via PE matmul against the preloaded M tiles, dense
    concat([h,x1]) @ W as feature-major fp16 matmuls with fused tanh+bias,
    PE-transpose + dinv-scale to produce the node-major fp16 hn table,
    AllGather in 2 halves so next-layer group-0 gathers overlap the second
    collective.
"""

import sys

sys.path.insert(0, "/opt/trn_rl_repo")

import numpy as np

import concourse.bacc as bacc
import concourse.bass as bass
import concourse.mybir as mybir
import concourse.tile as tile
from concourse.bass_utils import run_bass_kernel_spmd
from concourse.masks import make_identity
from concourse.tile import TileContext
from concourse.vector_clock import ScopedClock

F32 = mybir.dt.float32
F16 = mybir.dt.float16
AF = mybir.ActivationFunctionType

N_CORES = 8
KB = 1  # gather chunks per indirect-DMA instruction (HW supports 1 offset col)

# ---------------------------------------------------------------------------
# walrus on this image supports only ONE sync-wait command per instruction;
# Tile freely emits several.  Split extra waits onto same-engine NoOps.
# ---------------------------------------------------------------------------


def _patched_drain_and_barrier(self, tick_clock, wait_clock):
    nc = self.nc
    probe = nc.sync.nop(nofuse=True, hint="drain_wait_split")
    wait_clock.add_sem_waits(probe.ins, ScopedClock({None: tick_clock.global_clock}))
    si = probe.ins.sync_info
    waits = list(si.on_wait) if si is not None else []
    if si is not None:
        si.on_wait = []
    for w in waits:
        self._add_instruction(
            mybir.InstNoOp(
                name=nc.get_next_instruction_name(),
                engine=mybir.EngineType.SP,
                sync_info=mybir.SyncInfo(on_wait=[w], on_update=[]),
                bass_nofuse=True,
            )
        )
    nc.sync.drain()
    nc.all_engine_barrier()
    assert self.sems is not None
    popped = nc._tile_sem_poison_stack.pop()
    assert popped is self._sem_poison
    nc.clear_and_free_semaphores(list(self.sems.allocated().values()))
    nc.all_engine_barrier()


TileContext._drain_and_barrier = _patched_drain_and_barrier


def split_sync_waits(nc):
    for f in nc.m.functions:
        for blk in f.blocks:
            insts = blk.instructions
            if not any(
                i.sync_info is not None
                and i.sync_info.on_wait
                and len(i.sync_info.on_wait) > 1
                for i in insts
            ):
                continue
            new = []
            for inst in insts:
                si = inst.sync_info
                if si is not None and si.on_wait and len(si.on_wait) > 1:
                    waits = list(si.on_wait)
                    for w in waits[:-1]:
                        new.append(
                            mybir.InstNoOp(
                                name=nc.get_next_instruction_name(),
                                engine=inst.engine,
                                sync_info=mybir.SyncInfo(on_wait=[w], on_update=[]),
                                bass_nofuse=True,
                            )
                        )
                    si.on_wait = [waits[-1]]
                new.append(inst)
            blk.instructions = new


# ---------------------------------------------------------------------------
# Host-side plan
# ---------------------------------------------------------------------------


class Plan:
    pass


def _balance_perm(src, dst, N):
    """Per-core node placement balancing per-(tile, src-half) in-degree sums.

    The gather schedule is shared across cores and padded to the max per-cell
    edge count; flattening each core's per-tile (half1, half2) in-degree
    profile shrinks that max toward the mean (~4% fewer gather chunks).
    Each node keeps its core and its source-half (so per-edge half bits and
    the AllGather geometry are unchanged); only its tile slot moves.
    Returns perm: old node id -> new node id (position).
    """
    B = N // N_CORES
    H = B // 2
    g_src = ((src % B) >= H).astype(np.int64)
    deg1 = np.bincount(dst[g_src == 0], minlength=N).astype(np.int64)
    deg2 = np.bincount(dst[g_src == 1], minlength=N).astype(np.int64)

    g_node = (np.arange(N) % B) >= H

    # nodes may migrate across cores (all consumers are per-core data and the
    # output is unpermuted on host); only the source-half must be preserved so
    # the per-edge g bits and deg1/deg2 stay valid. Global bins balance core
    # totals too, pushing every (tile, half) cell to the global mean.
    perm = np.empty(N, np.int64)
    for half in range(2):
        nodes = np.nonzero(g_node == bool(half))[0]
        starts, caps = [], []
        for c in range(N_CORES):
            lo, hi = (0, H) if half == 0 else (H, B)
            pos = lo
            while pos < hi:
                w = min(128 - (pos % 128), hi - pos)
                starts.append(c * B + pos)
                caps.append(w)
                pos += w
        d1 = deg1[nodes].astype(np.float64)
        d2 = deg2[nodes].astype(np.float64)
        order = np.argsort(-(d1 + d2), kind="stable")
        nb = len(caps)
        s1 = np.zeros(nb)
        s2 = np.zeros(nb)
        fill = np.zeros(nb, np.int64)
        caps_a = np.asarray(caps, np.float64)
        starts_a = np.asarray(starts, np.int64)
        for i in order:
            # normalized so partial bins target proportionally smaller sums
            score = np.maximum(s1 + d1[i], s2 + d2[i]) / caps_a
            score = np.where(fill < caps_a, score, np.inf)
            b = int(np.argmin(score))
            perm[nodes[i]] = starts_a[b] + fill[b]
            fill[b] += 1
            s1[b] += d1[i]
            s2[b] += d2[i]
    return perm


def build_plan(x, src, dst, n_nodes):
    p = Plan()
    N = n_nodes

    perm = _balance_perm(np.asarray(src, np.int64), np.asarray(dst, np.int64), N)
    inv = np.argsort(perm)
    x = np.asarray(x)[inv]
    src = perm[np.asarray(src, np.int64)].astype(np.int32)
    dst = perm[np.asarray(dst, np.int64)].astype(np.int32)
    p.out_perm = perm

    B = N // N_CORES          # dst nodes per core
    H = B // 2                # rows per AllGather half
    T = -(-B // 128)          # dst tiles per core
    p.N, p.B, p.H, p.T = N, B, H, T
    p.last_w = B - (T - 1) * 128

    deg = np.bincount(dst, minlength=N).astype(np.float32)
    dinv = np.where(deg > 0, 1.0 / np.sqrt(np.maximum(deg, 1.0)), 0.0).astype(
        np.float32
    )
    p.dinv = dinv

    core = dst // B
    dl = dst % B
    t = dl // 128
    d = (dl % 128).astype(np.int64)
    sl = src % B
    g = (sl >= H).astype(np.int64)
    row = (src // B) * H + (sl - g * H)

    key = ((core.astype(np.int64) * T + t) * 2 + g).astype(np.int64)
    order = np.argsort(key, kind="stable")
    ks = key[order]
    cnt = np.bincount(key, minlength=N_CORES * T * 2).reshape(N_CORES, T, 2)

    # Shared (all-core) per-(tile,group) segment sizes; chunks of 128 edges
    # may SPAN tile boundaries within a group's stream (a spanning chunk is
    # gathered once and multiplied against two selection matrices).
    cntmax = np.maximum(cnt.max(axis=0), 1)        # [T, 2] edges
    seg_base = np.zeros((T, 2), np.int64)
    seg_base[1:, 0] = np.cumsum(cntmax[:, 0])[:-1]
    seg_base[1:, 1] = np.cumsum(cntmax[:, 1])[:-1]
    Lg = cntmax.sum(axis=0)                        # [2]
    Cg = -(-Lg // 128)
    p.Cg = [int(Cg[0]), int(Cg[1])]
    C = int(Cg.sum())
    p.C = C
    g_base = np.array([0, Cg[0]], np.int64)
    p.g_base = [0, int(Cg[0])]

    clo = seg_base // 128                          # first chunk touching (t,g)
    chi = (seg_base + cntmax - 1) // 128           # last chunk touching (t,g)
    novl = chi - clo + 1
    dbase = np.zeros((T, 2), np.int64)
    dbase.reshape(-1)[1:] = np.cumsum(novl.reshape(-1))[:-1]
    D = int(novl.sum())
    p.D = D
    p.maxnovl = int(novl.max())

    # within-(core,t,g) rank of each edge
    E = len(dst)
    starts = np.zeros(N_CORES * T * 2 + 1, np.int64)
    starts[1:] = np.cumsum(cnt.reshape(-1))
    rank = np.arange(E, dtype=np.int64) - starts[ks]

    idx = np.zeros((N_CORES, 128, C), np.int32)
    Mall = np.zeros((N_CORES, 128, D * 128), np.float16)

    oc = core[order]
    ot = t[order]
    og = g[order]
    pos = seg_base[ot, og] + rank
    chunk = pos // 128
    part = pos % 128
    gcol = g_base[og] + chunk
    dcol = dbase[ot, og] + (chunk - clo[ot, og])
    idx[oc, part, gcol] = row[order].astype(np.int32)
    Mall[oc, part, dcol * 128 + d[order]] = (-dinv[dst[order]]).astype(np.float16)
    p.idx, p.Mall = idx, Mall

    # device schedule: per (group, tile) -> [(chunk, dcol)]
    tile_items = [[], []]
    for gg in range(2):
        for tt in range(T):
            items = []
            for k, ch in enumerate(range(clo[tt, gg], chi[tt, gg] + 1)):
                items.append((int(ch), int(dbase[tt, gg] + k)))
            tile_items[gg].append(items)
    p.tile_items = tile_items

    # dinv in node-major tile columns, per core: [cores, 128, T]
    dinvc = np.zeros((N_CORES, 128, T), np.float32)
    vb = dinv.reshape(N_CORES, B)
    for tt in range(T):
        w = min(128, B - tt * 128)
        dinvc[:, :w, tt] = vb[:, tt * 128 : tt * 128 + w]
    p.dinvc = dinvc

    # layer-1 message passing is weight-independent: x1_1 = -dinv*(A @ (x*dinv))
    # computed on host (input preprocessing), shipped feature-major per core.
    hn1 = (x * dinv[:, None]).astype(np.float32)
    agg = np.zeros_like(hn1)
    np.add.at(agg, dst, hn1[src])
    x1_1 = (-(agg * dinv[:, None])).astype(np.float16)
    p.x1T = np.ascontiguousarray(
        x1_1.reshape(N_CORES, B, x.shape[1]).transpose(0, 2, 1)
    )

    # per-core feature-major x block
    p.xT = np.ascontiguousarray(
        x.astype(np.float16).reshape(N_CORES, B, x.shape[1]).transpose(0, 2, 1)
    )
    return p


# ---------------------------------------------------------------------------
# Device program
# ---------------------------------------------------------------------------


def build_nc(p):
    B, H, T, C, D = p.B, p.H, p.T, p.C, p.D
    NS = -(-B // 512)  # node slabs for dense matmul
    MW = p.maxnovl * 128

    nc = bacc.Bacc("TRN2")
    xT_in = nc.declare_dram_parameter("xT", [128, B], F16, isOutput=False)
    x1T_in = nc.declare_dram_parameter("x1T", [128, B], F16, isOutput=False)
    idx_in = nc.declare_dram_parameter("idx", [128, C], mybir.dt.int32, isOutput=False)
    M_in = nc.declare_dram_parameter("Mall", [128, D * 128], F16, isOutput=False)
    dinvc_in = nc.declare_dram_parameter("dinvc", [128, T], F32, isOutput=False)
    W1_in = nc.declare_dram_parameter("W1", [256, 256], F16, isOutput=False)
    W2_in = nc.declare_dram_parameter("W2", [512, 256], F16, isOutput=False)
    W3_in = nc.declare_dram_parameter("W3", [512, 128], F16, isOutput=False)
    b1_in = nc.declare_dram_parameter("b1", [256, 1], F32, isOutput=False)
    b2_in = nc.declare_dram_parameter("b2", [256, 1], F32, isOutput=False)
    b3_in = nc.declare_dram_parameter("b3", [128, 1], F32, isOutput=False)
    out_p = nc.declare_dram_parameter("outT", [128, B], F16, isOutput=True)

    hn_stage = nc.dram_tensor("hn_stage", [B, 256], F16)
    hn_sh1 = nc.dram_tensor("hn_sh1", [N_CORES * H, 256], F16, addr_space="Shared")
    hn_sh2 = nc.dram_tensor("hn_sh2", [N_CORES * H, 256], F16, addr_space="Shared")

    from contextlib import ExitStack

    with TileContext(nc) as tc, ExitStack() as es:
        cst = es.enter_context(tc.tile_pool(name="cst", bufs=1))
        gp = es.enter_context(tc.tile_pool(name="gp", bufs=24))
        ms = es.enter_context(tc.tile_pool(name="ms", bufs=6))
        evp = es.enter_context(tc.tile_pool(name="evp", bufs=4))
        hnp = es.enter_context(tc.tile_pool(name="hnp", bufs=4))
        agg_ps = es.enter_context(tc.tile_pool(name="agg_ps", bufs=4, space="PSUM"))
        tr_ps = es.enter_context(tc.tile_pool(name="tr_ps", bufs=2, space="PSUM"))
        dn_ps = es.enter_context(tc.tile_pool(name="dn_ps", bufs=2, space="PSUM"))

        # ---- constants ----
        idx_t = cst.tile([128, C], mybir.dt.int32, tag="idx")
        dinvc_t = cst.tile([128, T], F32, tag="dinvc")
        ident = cst.tile([128, 128], F16, tag="ident")
        nc.sync.dma_start(out=idx_t[:], in_=idx_in[:])
        nc.sync.dma_start(out=dinvc_t[:], in_=dinvc_in[:])
        make_identity(nc, ident[:])

        def load_w(w_in, K, FO):
            tiles = []
            for kk in range(K // 128):
                row = []
                for fo in range(FO // 128):
                    wt = cst.tile([128, 128], F16, tag=f"w{w_in.name}_{kk}_{fo}", name=f"w{w_in.name}_{kk}_{fo}")
                    nc.sync.dma_start(
                        out=wt[:],
                        in_=w_in[kk * 128 : (kk + 1) * 128, fo * 128 : (fo + 1) * 128],
                    )
                    row.append(wt)
                tiles.append(row)
            return tiles

        W1t = load_w(W1_in, 256, 256)
        W2t = load_w(W2_in, 512, 256)
        W3t = load_w(W3_in, 512, 128)
        bt = {}
        for name, b_in, FO in (("b1", b1_in, 256), ("b2", b2_in, 256), ("b3", b3_in, 128)):
            bt[name] = []
            for fo in range(FO // 128):
                btile = cst.tile([128, 1], F32, tag=f"{name}_{fo}", name=f"{name}_{fo}")
                nc.sync.dma_start(out=btile[:], in_=b_in[fo * 128 : (fo + 1) * 128, :])
                bt[name].append(btile)

        # ---- persistent activations ----
        hA = [cst.tile([128, B], F16, tag=f"hA{i}", name=f"hA{i}") for i in range(2)]
        hB = [cst.tile([128, B], F16, tag=f"hB{i}", name=f"hB{i}") for i in range(2)]
        x1 = [cst.tile([128, B], F16, tag=f"x1_{i}", name=f"x1_{i}") for i in range(2)]
        x1L1 = cst.tile([128, B], F16, tag="x1L1", name="x1L1")
        nc.sync.dma_start(out=hA[0][:], in_=xT_in[:])
        nc.sync.dma_start(out=x1L1[:], in_=x1T_in[:])

        layers = [
            (128, 256, W1t, bt["b1"], AF.Tanh),
            (256, 256, W2t, bt["b2"], AF.Tanh),
            (256, 256, W2t, bt["b2"], AF.Tanh),
            (256, 256, W2t, bt["b2"], AF.Tanh),
            (256, 128, W3t, bt["b3"], AF.Identity),
        ]

        cur, nxt = hA, hB
        for li, (FI, FO, Wt, bias, act) in enumerate(layers):
            tables = (hn_sh1, hn_sh2)
            nh = FI // 128

            # -------- gather + segment-sum + transpose --------
            # (layer 1's x1 is weight-independent and precomputed on host)
            for gg in range(2) if li > 0 else ():
                # batched gathers: KB chunks (KB*128 rows) per SWDGE instruction
                gtiles = []
                nb = -(-p.Cg[gg] // KB)
                for b in range(nb):
                    c0 = b * KB
                    kb = min(KB, p.Cg[gg] - c0)
                    gt = gp.tile([128, KB * FI], F16, tag="g")
                    nc.gpsimd.indirect_dma_start(
                        out=gt[:, : kb * FI],
                        out_offset=None,
                        in_=tables[gg][:],
                        in_offset=bass.IndirectOffsetOnAxis(
                            ap=idx_t[:, p.g_base[gg] + c0 : p.g_base[gg] + c0 + kb],
                            axis=0,
                        ),
                    )
                    gtiles.append(gt)

                for tt in range(T):
                    tw = min(128, B - tt * 128)
                    items = p.tile_items[gg][tt]
                    novl = len(items)
                    d0 = items[0][1]
                    Mt = ms.tile([128, MW], F16, tag="m")
                    nc.sync.dma_start(
                        out=Mt[:, : novl * 128],
                        in_=M_in[:, d0 * 128 : (d0 + novl) * 128],
                    )
                    agg = agg_ps.tile([128, FI], F32, tag="agg", space="PSUM")
                    for j, (ch, dcol) in enumerate(items):
                        b, off = divmod(ch, KB)
                        nc.tensor.matmul(
                            out=agg[:],
                            lhsT=Mt[:, (dcol - d0) * 128 : (dcol - d0 + 1) * 128],
                            rhs=gtiles[b][:, off * FI : (off + 1) * FI],
                            start=(j == 0),
                            stop=(j == novl - 1),
                        )
                    x1nm = evp.tile([128, 256], F16, tag="x1nm")
                    nc.vector.tensor_copy(out=x1nm[:, :FI], in_=agg[:])
                    for hh in range(nh):
                        trp = tr_ps.tile([128, 128], F16, tag="tr", space="PSUM")
                        nc.tensor.transpose(
                            out=trp[:],
                            in_=x1nm[:, hh * 128 : (hh + 1) * 128],
                            identity=ident[:],
                        )
                        if gg == 0:
                            nc.vector.tensor_copy(
                                out=x1[hh][:, tt * 128 : tt * 128 + tw],
                                in_=trp[:, :tw],
                            )
                        else:
                            nc.vector.tensor_add(
                                out=x1[hh][:, tt * 128 : tt * 128 + tw],
                                in0=x1[hh][:, tt * 128 : tt * 128 + tw],
                                in1=trp[:, :tw],
                            )

            # -------- dense: out = concat(h, x1) @ W + b --------
            x1_src = [x1L1] if li == 0 else x1
            rhs_list = [cur[i] for i in range(nh)] + [x1_src[i] for i in range(nh)]
            for s in range(NS):
                s0 = s * 512
                sw = min(512, B - s0)
                for fo in range(FO // 128):
                    dps = dn_ps.tile([128, 512], F32, tag="dn", space="PSUM")
                    for kk in range(2 * nh):
                        nc.tensor.matmul(
                            out=dps[:, :sw],
                            lhsT=Wt[kk][fo][:],
                            rhs=rhs_list[kk][:, s0 : s0 + sw],
                            start=(kk == 0),
                            stop=(kk == 2 * nh - 1),
                        )
                    nc.scalar.activation(
                        out=nxt[fo][:, s0 : s0 + sw],
                        in_=dps[:, :sw],
                        func=act,
                        bias=bias[fo][:],
                    )

            # -------- hn = h*dinv (node-major) + AllGather --------
            if li < len(layers) - 1:
                cc1_after = (H + 127) // 128 - 1  # last node-tile feeding half 1
                for tt in range(T):
                    tw = min(128, B - tt * 128)
                    hn_nm = hnp.tile([128, 256], F16, tag="hn_nm")
                    for hh in range(FO // 128):
                        trp = tr_ps.tile([128, 128], F16, tag="tr", space="PSUM")
                        nc.tensor.transpose(
                            out=trp[:tw, :],
                            in_=nxt[hh][:, tt * 128 : tt * 128 + tw],
                            identity=ident[:],
                        )
                        nc.vector.tensor_scalar(
                            out=hn_nm[:tw, hh * 128 : (hh + 1) * 128],
                            in0=trp[:tw, :],
                            scalar1=dinvc_t[:tw, tt : tt + 1],
                            scalar2=None,
                            op0=mybir.AluOpType.mult,
                        )
                    nc.sync.dma_start(
                        out=hn_stage[tt * 128 : tt * 128 + tw, :], in_=hn_nm[:tw, :]
                    )
                    if tt == cc1_after:
                        nc.gpsimd.collective_compute(
                            "AllGather",
                            mybir.AluOpType.bypass,
                            replica_groups=[list(range(N_CORES))],
                            ins=[hn_stage[0:H, :]],
                            outs=[hn_sh1[:]],
                        )
                nc.gpsimd.collective_compute(
                    "AllGather",
                    mybir.AluOpType.bypass,
                    replica_groups=[list(range(N_CORES))],
                    ins=[hn_stage[H:B, :]],
                    outs=[hn_sh2[:]],
                )
            cur, nxt = nxt, cur

        nc.sync.dma_start(out=out_p[:], in_=cur[0][:])

    nc.compile()
    split_sync_waits(nc)
    bass.Bass.finalize(nc)
    return nc


# ---------------------------------------------------------------------------
# Entry point
# ---------------------------------------------------------------------------


def make_in_maps(p, W1, b1, W2, b2, W3, b3):
    in_maps = []
    for c in range(N_CORES):
        in_maps.append(
            {
                "xT": p.xT[c],
                "x1T": p.x1T[c],
                "idx": p.idx[c],
                "Mall": p.Mall[c].reshape(128, -1),
                "dinvc": p.dinvc[c],
                "W1": np.asarray(W1, np.float16),
                "W2": np.asarray(W2, np.float16),
                "W3": np.asarray(W3, np.float16),
                "b1": np.asarray(b1, np.float32).reshape(-1, 1),
                "b2": np.asarray(b2, np.float32).reshape(-1, 1),
                "b3": np.asarray(b3, np.float32).reshape(-1, 1),
            }
        )
    return in_maps


def kernel(x, src, dst, W1, b1, W2, b2, W3, b3):
    x = np.asarray(x, np.float32)
    src = np.asarray(src, np.int32)
    dst = np.asarray(dst, np.int32)
    p = build_plan(x, src, dst, x.shape[0])
    nc = build_nc(p)

    in_maps = make_in_maps(p, W1, b1, W2, b2, W3, b3)
    res = run_bass_kernel_spmd(nc, in_maps, list(range(N_CORES))).results
    out = np.empty((x.shape[0], W3.shape[1]), np.float32)
    B = p.B
    for c in range(N_CORES):
        out[c * B : (c + 1) * B, :] = res[c]["outT"].T
    return out[p.out_perm]


# revision 18
# speedup vs baseline: 1.0772x; 1.0028x over previous
"""ChebConv (k=2, DGL-style, lambda_max=2) on 8 Trainium2 NeuronCores.

Strategy (graph/data parallel over destination nodes):
  - Host: degree/dinv, per-core edge sort into (dst-tile, src-half) groups,
    128-edge chunks (shared across cores, padded to the per-(tile,group) max);
    selection matrices M[k,d] = (edge k hits local dst d) * (-dinv[dst_k])
    are precomputed on host in fp16 and streamed to the device per dst tile.
  - Layer-1 message passing (weight-independent) is precomputed on host.
  - Device, layers 2-5: indirect-DMA gather of hn[src] rows (128 rows per
    SWDGE instruction -- the HW honors exactly one offset per partition),
    segment-sum via PE matmul against the preloaded M tiles, dense
    concat([h,x1]) @ W as feature-major fp16 matmuls with fused tanh+bias,
    PE-transpose + dinv-scale to produce the node-major fp16 hn table,
    AllGather in 2 halves so next-layer group-0 gathers overlap the second
    collective.
"""

import sys

sys.path.insert(0, "/opt/trn_rl_repo")

import numpy as np

import concourse.bacc as bacc
import concourse.bass as bass
import concourse.mybir as mybir
import concourse.tile as tile
from concourse.bass_utils import run_bass_kernel_spmd
from concourse.masks import make_identity
from concourse.tile import TileContext
from concourse.vector_clock import ScopedClock

F32 = mybir.dt.float32
F16 = mybir.dt.float16
AF = mybir.ActivationFunctionType

N_CORES = 8
KB = 1  # gather chunks per indirect-DMA instruction (HW supports 1 offset col)

# ---------------------------------------------------------------------------
# walrus on this image supports only ONE sync-wait command per instruction;
# Tile freely emits several.  Split extra waits onto same-engine NoOps.
# ---------------------------------------------------------------------------


def _patched_drain_and_barrier(self, tick_clock, wait_clock):
    nc = self.nc
    probe = nc.sync.nop(nofuse=True, hint="drain_wait_split")
    wait_clock.add_sem_waits(probe.ins, ScopedClock({None: tick_clock.global_clock}))
    si = probe.ins.sync_info
    waits = list(si.on_wait) if si is not None else []
    if si is not None:
        si.on_wait = []
    for w in waits:
        self._add_instruction(
            mybir.InstNoOp(
                name=nc.get_next_instruction_name(),
                engine=mybir.EngineType.SP,
                sync_info=mybir.SyncInfo(on_wait=[w], on_update=[]),
                bass_nofuse=True,
            )
        )
    nc.sync.drain()
    nc.all_engine_barrier()
    assert self.sems is not None
    popped = nc._tile_sem_poison_stack.pop()
    assert popped is self._sem_poison
    nc.clear_and_free_semaphores(list(self.sems.allocated().values()))
    nc.all_engine_barrier()


TileContext._drain_and_barrier = _patched_drain_and_barrier


def split_sync_waits(nc):
    for f in nc.m.functions:
        for blk in f.blocks:
            insts = blk.instructions
            if not any(
                i.sync_info is not None
                and i.sync_info.on_wait
                and len(i.sync_info.on_wait) > 1
                for i in insts
            ):
                continue
            new = []
            for inst in insts:
                si = inst.sync_info
                if si is not None and si.on_wait and len(si.on_wait) > 1:
                    waits = list(si.on_wait)
                    for w in waits[:-1]:
                        new.append(
                            mybir.InstNoOp(
                                name=nc.get_next_instruction_name(),
                                engine=inst.engine,
                                sync_info=mybir.SyncInfo(on_wait=[w], on_update=[]),
                                bass_nofuse=True,
                            )
                        )
                    si.on_wait = [waits[-1]]
                new.append(inst)
            blk.instructions = new


# ---------------------------------------------------------------------------
# Host-side plan
# ---------------------------------------------------------------------------


class Plan:
    pass


def _balance_perm(src, dst, N):
    """Global node placement balancing per-(tile, src-half) in-degree sums.

    The gather schedule is shared across cores and padded to the max per-cell
    edge count; a 2-D LPT greedy over ALL 8 cores' tile bins pushes every
    cell's (half1, half2) in-degree sums to the global mean (~4% fewer gather
    chunks than the natural layout). Nodes may migrate cores (all consumers
    are per-core inputs; the output is unpermuted on host) but keep their
    source-half so per-edge half bits and the AllGather geometry are unchanged.
    Returns perm: old node id -> new node id (position).
    """
    B = N // N_CORES
    H = B // 2
    g_src = ((src % B) >= H).astype(np.int64)
    deg1 = np.bincount(dst[g_src == 0], minlength=N).astype(np.int64)
    deg2 = np.bincount(dst[g_src == 1], minlength=N).astype(np.int64)

    g_node = (np.arange(N) % B) >= H

    # nodes may migrate across cores (all consumers are per-core data and the
    # output is unpermuted on host); only the source-half must be preserved so
    # the per-edge g bits and deg1/deg2 stay valid. Global bins balance core
    # totals too, pushing every (tile, half) cell to the global mean.
    perm = np.empty(N, np.int64)
    for half in range(2):
        nodes = np.nonzero(g_node == bool(half))[0]
        starts, caps = [], []
        for c in range(N_CORES):
            lo, hi = (0, H) if half == 0 else (H, B)
            pos = lo
            while pos < hi:
                w = min(128 - (pos % 128), hi - pos)
                starts.append(c * B + pos)
                caps.append(w)
                pos += w
        d1 = deg1[nodes].astype(np.float64)
        d2 = deg2[nodes].astype(np.float64)
        order = np.argsort(-(d1 + d2), kind="stable")
        nb = len(caps)
        s1 = np.zeros(nb)
        s2 = np.zeros(nb)
        fill = np.zeros(nb, np.int64)
        caps_a = np.asarray(caps, np.float64)
        starts_a = np.asarray(starts, np.int64)
        for i in order:
            # normalized so partial bins target proportionally smaller sums
            score = np.maximum(s1 + d1[i], s2 + d2[i]) / caps_a
            score = np.where(fill < caps_a, score, np.inf)
            b = int(np.argmin(score))
            perm[nodes[i]] = starts_a[b] + fill[b]
            fill[b] += 1
            s1[b] += d1[i]
            s2[b] += d2[i]
    return perm


def build_plan(x, src, dst, n_nodes):
    p = Plan()
    N = n_nodes

    perm = _balance_perm(np.asarray(src, np.int64), np.asarray(dst, np.int64), N)
    inv = np.argsort(perm)
    x = np.asarray(x)[inv]
    src = perm[np.asarray(src, np.int64)].astype(np.int32)
    dst = perm[np.asarray(dst, np.int64)].astype(np.int32)
    p.out_perm = perm

    B = N // N_CORES          # dst nodes per core
    H = B // 2                # rows per AllGather half
    T = -(-B // 128)          # dst tiles per core
    p.N, p.B, p.H, p.T = N, B, H, T
    p.last_w = B - (T - 1) * 128

    deg = np.bincount(dst, minlength=N).astype(np.float32)
    dinv = np.where(deg > 0, 1.0 / np.sqrt(np.maximum(deg, 1.0)), 0.0).astype(
        np.float32
    )
    p.dinv = dinv

    core = dst // B
    dl = dst % B
    t = dl // 128
    d = (dl % 128).astype(np.int64)
    sl = src % B
    g = (sl >= H).astype(np.int64)
    row = (src // B) * H + (sl - g * H)

    key = ((core.astype(np.int64) * T + t) * 2 + g).astype(np.int64)
    order = np.argsort(key, kind="stable")
    ks = key[order]
    cnt = np.bincount(key, minlength=N_CORES * T * 2).reshape(N_CORES, T, 2)

    # Shared (all-core) per-(tile,group) segment sizes; chunks of 128 edges
    # may SPAN tile boundaries within a group's stream (a spanning chunk is
    # gathered once and multiplied against two selection matrices).
    cntmax = np.maximum(cnt.max(axis=0), 1)        # [T, 2] edges
    seg_base = np.zeros((T, 2), np.int64)
    seg_base[1:, 0] = np.cumsum(cntmax[:, 0])[:-1]
    seg_base[1:, 1] = np.cumsum(cntmax[:, 1])[:-1]
    Lg = cntmax.sum(axis=0)                        # [2]
    Cg = -(-Lg // 128)
    p.Cg = [int(Cg[0]), int(Cg[1])]
    C = int(Cg.sum())
    p.C = C
    g_base = np.array([0, Cg[0]], np.int64)
    p.g_base = [0, int(Cg[0])]

    clo = seg_base // 128                          # first chunk touching (t,g)
    chi = (seg_base + cntmax - 1) // 128           # last chunk touching (t,g)
    novl = chi - clo + 1
    dbase = np.zeros((T, 2), np.int64)
    dbase.reshape(-1)[1:] = np.cumsum(novl.reshape(-1))[:-1]
    D = int(novl.sum())
    p.D = D
    p.maxnovl = int(novl.max())

    # within-(core,t,g) rank of each edge
    E = len(dst)
    starts = np.zeros(N_CORES * T * 2 + 1, np.int64)
    starts[1:] = np.cumsum(cnt.reshape(-1))
    rank = np.arange(E, dtype=np.int64) - starts[ks]

    idx = np.zeros((N_CORES, 128, C), np.int32)
    Mall = np.zeros((N_CORES, 128, D * 128), np.float16)

    oc = core[order]
    ot = t[order]
    og = g[order]
    pos = seg_base[ot, og] + rank
    chunk = pos // 128
    part = pos % 128
    gcol = g_base[og] + chunk
    dcol = dbase[ot, og] + (chunk - clo[ot, og])
    idx[oc, part, gcol] = row[order].astype(np.int32)
    Mall[oc, part, dcol * 128 + d[order]] = (-dinv[dst[order]]).astype(np.float16)
    p.idx, p.Mall = idx, Mall

    # device schedule: per (group, tile) -> [(chunk, dcol)]
    tile_items = [[], []]
    for gg in range(2):
        for tt in range(T):
            items = []
            for k, ch in enumerate(range(clo[tt, gg], chi[tt, gg] + 1)):
                items.append((int(ch), int(dbase[tt, gg] + k)))
            tile_items[gg].append(items)
    p.tile_items = tile_items

    # dinv in node-major tile columns, per core: [cores, 128, T]
    dinvc = np.zeros((N_CORES, 128, T), np.float32)
    vb = dinv.reshape(N_CORES, B)
    for tt in range(T):
        w = min(128, B - tt * 128)
        dinvc[:, :w, tt] = vb[:, tt * 128 : tt * 128 + w]
    p.dinvc = dinvc

    # layer-1 message passing is weight-independent: x1_1 = -dinv*(A @ (x*dinv))
    # computed on host (input preprocessing), shipped feature-major per core.
    hn1 = (x * dinv[:, None]).astype(np.float32)
    agg = np.zeros_like(hn1)
    np.add.at(agg, dst, hn1[src])
    x1_1 = (-(agg * dinv[:, None])).astype(np.float16)
    p.x1T = np.ascontiguousarray(
        x1_1.reshape(N_CORES, B, x.shape[1]).transpose(0, 2, 1)
    )

    # per-core feature-major x block
    p.xT = np.ascontiguousarray(
        x.astype(np.float16).reshape(N_CORES, B, x.shape[1]).transpose(0, 2, 1)
    )
    return p


# ---------------------------------------------------------------------------
# Device program
# ---------------------------------------------------------------------------


def build_nc(p):
    B, H, T, C, D = p.B, p.H, p.T, p.C, p.D
    NS = -(-B // 512)  # node slabs for dense matmul
    MW = p.maxnovl * 128

    nc = bacc.Bacc("TRN2")
    xT_in = nc.declare_dram_parameter("xT", [128, B], F16, isOutput=False)
    x1T_in = nc.declare_dram_parameter("x1T", [128, B], F16, isOutput=False)
    idx_in = nc.declare_dram_parameter("idx", [128, C], mybir.dt.int32, isOutput=False)
    M_in = nc.declare_dram_parameter("Mall", [128, D * 128], F16, isOutput=False)
    dinvc_in = nc.declare_dram_parameter("dinvc", [128, T], F32, isOutput=False)
    W1_in = nc.declare_dram_parameter("W1", [256, 256], F16, isOutput=False)
    W2_in = nc.declare_dram_parameter("W2", [512, 256], F16, isOutput=False)
    W3_in = nc.declare_dram_parameter("W3", [512, 128], F16, isOutput=False)
    b1_in = nc.declare_dram_parameter("b1", [256, 1], F32, isOutput=False)
    b2_in = nc.declare_dram_parameter("b2", [256, 1], F32, isOutput=False)
    b3_in = nc.declare_dram_parameter("b3", [128, 1], F32, isOutput=False)
    out_p = nc.declare_dram_parameter("outT", [128, B], F16, isOutput=True)

    hn_stage = nc.dram_tensor("hn_stage", [B, 256], F16)
    hn_sh1 = nc.dram_tensor("hn_sh1", [N_CORES * H, 256], F16, addr_space="Shared")
    hn_sh2 = nc.dram_tensor("hn_sh2", [N_CORES * H, 256], F16, addr_space="Shared")

    from contextlib import ExitStack

    with TileContext(nc) as tc, ExitStack() as es:
        cst = es.enter_context(tc.tile_pool(name="cst", bufs=1))
        gp = es.enter_context(tc.tile_pool(name="gp", bufs=24))
        ms = es.enter_context(tc.tile_pool(name="ms", bufs=6))
        evp = es.enter_context(tc.tile_pool(name="evp", bufs=4))
        hnp = es.enter_context(tc.tile_pool(name="hnp", bufs=4))
        agg_ps = es.enter_context(tc.tile_pool(name="agg_ps", bufs=4, space="PSUM"))
        tr_ps = es.enter_context(tc.tile_pool(name="tr_ps", bufs=2, space="PSUM"))
        dn_ps = es.enter_context(tc.tile_pool(name="dn_ps", bufs=2, space="PSUM"))

        # ---- constants ----
        idx_t = cst.tile([128, C], mybir.dt.int32, tag="idx")
        dinvc_t = cst.tile([128, T], F32, tag="dinvc")
        ident = cst.tile([128, 128], F16, tag="ident")
        nc.sync.dma_start(out=idx_t[:], in_=idx_in[:])
        nc.sync.dma_start(out=dinvc_t[:], in_=dinvc_in[:])
        make_identity(nc, ident[:])

        def load_w(w_in, K, FO):
            tiles = []
            for kk in range(K // 128):
                row = []
                for fo in range(FO // 128):
                    wt = cst.tile([128, 128], F16, tag=f"w{w_in.name}_{kk}_{fo}", name=f"w{w_in.name}_{kk}_{fo}")
                    nc.sync.dma_start(
                        out=wt[:],
                        in_=w_in[kk * 128 : (kk + 1) * 128, fo * 128 : (fo + 1) * 128],
                    )
                    row.append(wt)
                tiles.append(row)
            return tiles

        W1t = load_w(W1_in, 256, 256)
        W2t = load_w(W2_in, 512, 256)
        W3t = load_w(W3_in, 512, 128)
        bt = {}
        for name, b_in, FO in (("b1", b1_in, 256), ("b2", b2_in, 256), ("b3", b3_in, 128)):
            bt[name] = []
            for fo in range(FO // 128):
                btile = cst.tile([128, 1], F32, tag=f"{name}_{fo}", name=f"{name}_{fo}")
                nc.sync.dma_start(out=btile[:], in_=b_in[fo * 128 : (fo + 1) * 128, :])
                bt[name].append(btile)

        # ---- persistent activations ----
        hA = [cst.tile([128, B], F16, tag=f"hA{i}", name=f"hA{i}") for i in range(2)]
        hB = [cst.tile([128, B], F16, tag=f"hB{i}", name=f"hB{i}") for i in range(2)]
        x1 = [cst.tile([128, B], F16, tag=f"x1_{i}", name=f"x1_{i}") for i in range(2)]
        x1L1 = cst.tile([128, B], F16, tag="x1L1", name="x1L1")
        nc.sync.dma_start(out=hA[0][:], in_=xT_in[:])
        nc.sync.dma_start(out=x1L1[:], in_=x1T_in[:])

        layers = [
            (128, 256, W1t, bt["b1"], AF.Tanh),
            (256, 256, W2t, bt["b2"], AF.Tanh),
            (256, 256, W2t, bt["b2"], AF.Tanh),
            (256, 256, W2t, bt["b2"], AF.Tanh),
            (256, 128, W3t, bt["b3"], AF.Identity),
        ]

        cur, nxt = hA, hB
        for li, (FI, FO, Wt, bias, act) in enumerate(layers):
            tables = (hn_sh1, hn_sh2)
            nh = FI // 128

            # -------- gather + segment-sum + transpose --------
            # (layer 1's x1 is weight-independent and precomputed on host)
            for gg in range(2) if li > 0 else ():
                # batched gathers: KB chunks (KB*128 rows) per SWDGE instruction
                gtiles = []
                nb = -(-p.Cg[gg] // KB)
                for b in range(nb):
                    c0 = b * KB
                    kb = min(KB, p.Cg[gg] - c0)
                    gt = gp.tile([128, KB * FI], F16, tag="g")
                    nc.gpsimd.indirect_dma_start(
                        out=gt[:, : kb * FI],
                        out_offset=None,
                        in_=tables[gg][:],
                        in_offset=bass.IndirectOffsetOnAxis(
                            ap=idx_t[:, p.g_base[gg] + c0 : p.g_base[gg] + c0 + kb],
                            axis=0,
                        ),
                    )
                    gtiles.append(gt)

                for tt in range(T):
                    tw = min(128, B - tt * 128)
                    items = p.tile_items[gg][tt]
                    novl = len(items)
                    d0 = items[0][1]
                    Mt = ms.tile([128, MW], F16, tag="m")
                    nc.sync.dma_start(
                        out=Mt[:, : novl * 128],
                        in_=M_in[:, d0 * 128 : (d0 + novl) * 128],
                    )
                    agg = agg_ps.tile([128, FI], F32, tag="agg", space="PSUM")
                    for j, (ch, dcol) in enumerate(items):
                        b, off = divmod(ch, KB)
                        nc.tensor.matmul(
                            out=agg[:],
                            lhsT=Mt[:, (dcol - d0) * 128 : (dcol - d0 + 1) * 128],
                            rhs=gtiles[b][:, off * FI : (off + 1) * FI],
                            start=(j == 0),
                            stop=(j == novl - 1),
                        )
                    x1nm = evp.tile([128, 256], F16, tag="x1nm")
                    nc.vector.tensor_copy(out=x1nm[:, :FI], in_=agg[:])
                    for hh in range(nh):
                        trp = tr_ps.tile([128, 128], F16, tag="tr", space="PSUM")
                        nc.tensor.transpose(
                            out=trp[:],
                            in_=x1nm[:, hh * 128 : (hh + 1) * 128],
                            identity=ident[:],
                        )
                        if gg == 0:
                            nc.vector.tensor_copy(
                                out=x1[hh][:, tt * 128 : tt * 128 + tw],
                                in_=trp[:, :tw],
                            )
                        else:
                            nc.vector.tensor_add(
                                out=x1[hh][:, tt * 128 : tt * 128 + tw],
                                in0=x1[hh][:, tt * 128 : tt * 128 + tw],
                                in1=trp[:, :tw],
                            )

            # -------- dense: out = concat(h, x1) @ W + b --------
            x1_src = [x1L1] if li == 0 else x1
            rhs_list = [cur[i] for i in range(nh)] + [x1_src[i] for i in range(nh)]
            for s in range(NS):
                s0 = s * 512
                sw = min(512, B - s0)
                for fo in range(FO // 128):
                    dps = dn_ps.tile([128, 512], F32, tag="dn", space="PSUM")
                    for kk in range(2 * nh):
                        nc.tensor.matmul(
                            out=dps[:, :sw],
                            lhsT=Wt[kk][fo][:],
                            rhs=rhs_list[kk][:, s0 : s0 + sw],
                            start=(kk == 0),
                            stop=(kk == 2 * nh - 1),
                        )
                    nc.scalar.activation(
                        out=nxt[fo][:, s0 : s0 + sw],
                        in_=dps[:, :sw],
                        func=act,
                        bias=bias[fo][:],
                    )

            # -------- hn = h*dinv (node-major) + AllGather --------
            if li < len(layers) - 1:
                cc1_after = (H + 127) // 128 - 1  # last node-tile feeding half 1
                for tt in range(T):
                    tw = min(128, B - tt * 128)
                    hn_nm = hnp.tile([128, 256], F16, tag="hn_nm")
                    for hh in range(FO // 128):
                        trp = tr_ps.tile([128, 128], F16, tag="tr", space="PSUM")
                        nc.tensor.transpose(
                            out=trp[:tw, :],
                            in_=nxt[hh][:, tt * 128 : tt * 128 + tw],
                            identity=ident[:],
                        )
                        nc.vector.tensor_scalar(
                            out=hn_nm[:tw, hh * 128 : (hh + 1) * 128],
                            in0=trp[:tw, :],
                            scalar1=dinvc_t[:tw, tt : tt + 1],
                            scalar2=None,
                            op0=mybir.AluOpType.mult,
                        )
                    nc.sync.dma_start(
                        out=hn_stage[tt * 128 : tt * 128 + tw, :], in_=hn_nm[:tw, :]
                    )
                    if tt == cc1_after:
                        nc.gpsimd.collective_compute(
                            "AllGather",
                            mybir.AluOpType.bypass,
                            replica_groups=[list(range(N_CORES))],
                            ins=[hn_stage[0:H, :]],
                            outs=[hn_sh1[:]],
                        )
                nc.gpsimd.collective_compute(
                    "AllGather",
                    mybir.AluOpType.bypass,
                    replica_groups=[list(range(N_CORES))],
                    ins=[hn_stage[H:B, :]],
                    outs=[hn_sh2[:]],
                )
            cur, nxt = nxt, cur

        nc.sync.dma_start(out=out_p[:], in_=cur[0][:])

    nc.compile()
    split_sync_waits(nc)
    bass.Bass.finalize(nc)
    return nc


# ---------------------------------------------------------------------------
# Entry point
# ---------------------------------------------------------------------------


def make_in_maps(p, W1, b1, W2, b2, W3, b3):
    in_maps = []
    for c in range(N_CORES):
        in_maps.append(
            {
                "xT": p.xT[c],
                "x1T": p.x1T[c],
                "idx": p.idx[c],
                "Mall": p.Mall[c].reshape(128, -1),
                "dinvc": p.dinvc[c],
                "W1": np.asarray(W1, np.float16),
                "W2": np.asarray(W2, np.float16),
                "W3": np.asarray(W3, np.float16),
                "b1": np.asarray(b1, np.float32).reshape(-1, 1),
                "b2": np.asarray(b2, np.float32).reshape(-1, 1),
                "b3": np.asarray(b3, np.float32).reshape(-1, 1),
            }
        )
    return in_maps


def kernel(x, src, dst, W1, b1, W2, b2, W3, b3):
    x = np.asarray(x, np.float32)
    src = np.asarray(src, np.int32)
    dst = np.asarray(dst, np.int32)
    p = build_plan(x, src, dst, x.shape[0])
    nc = build_nc(p)

    in_maps = make_in_maps(p, W1, b1, W2, b2, W3, b3)
    res = run_bass_kernel_spmd(nc, in_maps, list(range(N_CORES))).results
    out = np.empty((x.shape[0], W3.shape[1]), np.float32)
    B = p.B
    for c in range(N_CORES):
        out[c * B : (c + 1) * B, :] = res[c]["outT"].T
    return out[p.out_perm]
